# revision 12
# baseline (speedup 1.0000x reference)
"""Trainium2 Bass kernel for nn_DeepFCNet (B=32 subjects, N=264 nets, D=375).

Strategy (8 NeuronCores, SPMD single NEFF):
  - Subject-data-parallel front: core k owns subjects 4k..4k+3. Feature
    extractor + similarity MLP for all 34716 pairs, 4-subject-packed into
    128 partitions via block-diagonal similarity weights (bf16 matmuls).
  - Pair layer 1 never materializes [P, 64]: with A = feats@sm_w1[:32]+b1
    and Bm = feats@sm_w1[32:], pre-activation of pair (i,j) is
    A[:, i] + Bm[:, j]; triu blocks make both reads affine (one DVE
    tensor_scalar per block segment). Blocks are padded to even column
    starts/lengths and Bm is kept in two one-column-shifted copies so
    every DVE op is 4-byte aligned (DVE 4x mode).
  - s ([4 x padded-pairs] tanh, bf16) is AllGathered in 4 chunks, batch
    DMA-transposed ([32, CH] -> [128, CH/128, 32]) and fed to a
    column-sharded classifier GEMM (cl_w1 column shard, bf16, padded rows
    zero). cl_b1 rides as an extra W row against a constant s=1 pad col.
  - c1 [32, 512] is transposed + rrelu'd locally (bf16); the first tail
    GEMM is computed on local columns only and AllReduced ([32, 256] fp32,
    cl_b2/8 folded per core); the rest of the tail (256-64-3 + log_softmax)
    runs redundantly on every core. End-to-end rel err ~2e-6.
"""

import os
import sys

for _p in ("/opt/trn_rl_repo", "/root/.axon_site/_ro/trn_rl_repo"):
    if os.path.isdir(_p) and _p not in sys.path:
        sys.path.insert(0, _p)

import numpy as np
import ml_dtypes

# ---------------------------------------------------------------- constants
NCORES = 8
B, NNET, D = 32, 264, 375
SPC = B // NCORES            # subjects per core = 4
DP = 384                     # padded feature dim (3 k-tiles)
INST = SPC * NNET            # 1056 instances per core through the FE
PAIRS = NNET * (NNET - 1) // 2   # 34716
COLS = 4096 // NCORES        # 512 classifier columns per core
SLOPE = 11.0 / 48.0
WB = 4                       # W k-tiles per DMA
W_BUFS = 24
NCHUNK = 4

_CACHE = {}
_last = {}


def _padded_blocks():
    """Blocks with even column starts and even padded lengths.
    Returns (blocks, used_cols): blocks = [(i, start, L, Lpad)]."""
    col, blocks = 0, []
    for i in range(NNET - 1):
        L = NNET - 1 - i
        Lp = L + (L & 1)
        blocks.append((i, col, L, Lp))
        col += Lp
    return blocks, col


_BLOCKS, _USED = _padded_blocks()        # used = 34848
PP = (_USED + 511) // 512 * 512          # padded pair columns = 35328
NT = PP // 512                           # 69 tiles
KT = PP // 128                           # 276 classifier k-tiles
_TILES_PER_CHUNK = [19, 19, 19, 12]
_TILE0 = [0, 19, 38, 57]
CH = [512 * t for t in _TILES_PER_CHUNK]          # chunk pair cols
KPC = [4 * t for t in _TILES_PER_CHUNK]           # chunk k-tiles
BIAS_COL = _USED                         # s=1 / cl_b1 row lives here


def _segments_by_tile():
    """Per 512-wide pair tile: list of (a, b, i, j0) meaning
    h1[:, a:b] = relu(Bm[:, j0 : j0+(b-a)] + A[:, i]). a, b-a even."""
    segs = [[] for _ in range(NT)]
    for (i, start, L, Lp) in _BLOCKS:
        s0, s1 = start, start + Lp
        for T in range(s0 // 512, (s1 - 1) // 512 + 1):
            lo, hi = max(s0, 512 * T), min(s1, 512 * (T + 1))
            segs[T].append((lo - 512 * T, hi - 512 * T, i, i + 1 + (lo - s0)))
    return segs


def _build():
    import concourse.tile as tile
    from concourse import bacc, mybir
    from concourse.masks import make_identity

    F32 = mybir.dt.float32
    BF16 = mybir.dt.bfloat16
    AF = mybir.ActivationFunctionType
    OP = mybir.AluOpType

    nc = bacc.Bacc(None, target_bir_lowering=False, debug=False,
                   num_devices=NCORES)
    segs_by_tile = _segments_by_tile()
    RG = [list(range(NCORES))]

    with tile.TileContext(nc) as tc:
        with tc.tile_pool(name="dram", bufs=1, space="DRAM") as dram:
            def din(name, shape, dt=F32):
                return dram.tile(shape, dt, kind="ExternalInput", name=name,
                                 uniquify=False)

            xT = din("xT", [DP, INST], BF16)
            fe_w1 = din("fe_w1", [DP, 64], BF16)
            fe_b1 = din("fe_b1", [64, 1])
            fe_w2 = din("fe_w2", [64, 64], BF16)
            fe_b2 = din("fe_b2", [64, 1])
            fe_w3 = din("fe_w3", [64, 32], BF16)
            fe_b3 = din("fe_b3", [32, 1])
            smA = din("smA", [32, 32], BF16)
            smB = din("smB", [32, 32], BF16)
            smb1t = din("smb1t", [128, 1])
            w2bd = din("w2bd", [128, 64], BF16)
            b2bdt = din("b2bdt", [64, 1])
            w3bd = din("w3bd", [64, 32], BF16)
            b3bdt = din("b3bdt", [32, 1])
            w4bd = din("w4bd", [32, 4], BF16)
            b4bdt = din("b4bdt", [4, 1])
            w1loc = din("w1loc", [PP, COLS], BF16)
            w2loc = din("w2loc", [640, 256], BF16)   # local-col rows + b2/8
            w3aug = din("w3aug", [384, 64])
            w4aug = din("w4aug", [128, 3])
            out = dram.tile([B, 3], F32, kind="ExternalOutput", name="out",
                            uniquify=False)

            warm_in = dram.tile([8, 8], F32, name="warm_in")
            warm_out = dram.tile([64, 8], F32, addr_space="Shared",
                                 name="warm_out")
            s_loc = [dram.tile([SPC, CH[c]], BF16, name=f"s_loc{c}")
                     for c in range(NCHUNK)]
            s_full = [dram.tile([B, CH[c]], BF16, addr_space="Shared",
                                name=f"s_full{c}") for c in range(NCHUNK)]
            ar_in = dram.tile([B, 256], F32, name="ar_in")
            ar_out = dram.tile([B, 256], F32, addr_space="Shared",
                               name="ar_out")

            with tc.tile_pool(name="const", bufs=1) as const:
                # fire a tiny collective first: absorbs first-call setup
                nc.gpsimd.collective_compute(
                    "AllGather", OP.bypass, replica_groups=RG,
                    ins=[warm_in[:]], outs=[warm_out[:]])

                def ld(name, shape, src, dt=F32):
                    t = const.tile(shape, dt, name=name)
                    nc.scalar.dma_start(t[:], src)
                    return t

                fe_w1_sb = ld("fe_w1_sb", [128, 3, 64],
                              fe_w1.rearrange("(t p) m -> p t m", p=128), BF16)
                fe_b1_sb = ld("fe_b1_sb", [64, 1], fe_b1[:])
                fe_w2_sb = ld("fe_w2_sb", [64, 64], fe_w2[:], BF16)
                fe_b2_sb = ld("fe_b2_sb", [64, 1], fe_b2[:])
                fe_w3_sb = ld("fe_w3_sb", [64, 32], fe_w3[:], BF16)
                fe_b3_sb = ld("fe_b3_sb", [32, 1], fe_b3[:])
                smA_sb = ld("smA_sb", [32, 32], smA[:], BF16)
                smB_sb = ld("smB_sb", [32, 32], smB[:], BF16)
                smb1_sb = ld("smb1_sb", [128, 1], smb1t[:])
                w2bd_sb = ld("w2bd_sb", [128, 64], w2bd[:], BF16)
                b2bd_sb = ld("b2bd_sb", [64, 1], b2bdt[:])
                w3bd_sb = ld("w3bd_sb", [64, 32], w3bd[:], BF16)
                b3bd_sb = ld("b3bd_sb", [32, 1], b3bdt[:])
                w4bd_sb = ld("w4bd_sb", [32, 4], w4bd[:], BF16)
                b4bd_sb = ld("b4bd_sb", [4, 1], b4bdt[:])
                w2loc_sb = ld("w2loc_sb", [128, 5, 256],
                              w2loc.rearrange("(t p) m -> p t m", p=128), BF16)
                w3aug_sb = ld("w3aug_sb", [128, 3, 64],
                              w3aug.rearrange("(t p) m -> p t m", p=128))
                w4aug_sb = ld("w4aug_sb", [128, 3], w4aug[:])
                ident_sb = const.tile([32, 32], F32, name="ident_sb")
                make_identity(nc, ident_sb[:])
                # pre-load ACT tables (Tanh/Relu/Exp/Ln) off the critical path
                scr = const.tile([32, 4], F32, name="scr")
                nc.scalar.activation(scr[:, 0:1], ident_sb[:, 0:1], AF.Tanh)
                nc.scalar.activation(scr[:, 1:2], ident_sb[:, 0:1], AF.Relu)
                nc.scalar.activation(scr[:, 2:3], ident_sb[:, 0:1], AF.Exp)
                nc.scalar.activation(scr[:, 3:4], scr[:, 2:3], AF.Ln)
                nc.sync.dma_start(warm_in[0:4, 0:4],
                                  scr[0:4, :])
                A_sb = const.tile([128, NNET], F32)      # f32: scalar operand
                B_ev = const.tile([128, 266], BF16)
                B_od = const.tile([128, 266], BF16)

                # ---------------- feature extractor ------------------------
                with (
                    nc.named_scope("fe"),
                    tc.tile_pool(name="fe", bufs=1) as fe,
                    tc.tile_pool(name="fetmp", bufs=2) as fetmp,
                    tc.tile_pool(name="feps", bufs=2, space="PSUM") as feps,
                ):
                    xT_sb = fe.tile([128, 3, INST], BF16)
                    nc.scalar.dma_start(xT_sb[:],
                                        xT.rearrange("(t p) i -> p t i",
                                                     p=128))
                    h1T = fe.tile([64, INST], BF16)
                    h2T = fe.tile([64, INST], BF16)
                    featsT = fe.tile([32, INST], BF16)
                    FC = INST // 3  # 352
                    for fc in range(3):
                        sl = slice(FC * fc, FC * (fc + 1))
                        p = feps.tile([64, FC], F32, tag="p64")
                        for t in range(3):
                            nc.tensor.matmul(p[:], fe_w1_sb[:, t, :],
                                             xT_sb[:, t, sl],
                                             start=(t == 0), stop=(t == 2))
                        tmp = fetmp.tile([64, FC], F32)
                        nc.vector.tensor_scalar(tmp[:], p[:], fe_b1_sb[:],
                                                None, OP.add)
                        nc.vector.scalar_tensor_tensor(
                            h1T[:, sl], tmp[:], SLOPE, tmp[:], OP.mult, OP.max)
                    for fc in range(3):
                        sl = slice(FC * fc, FC * (fc + 1))
                        p = feps.tile([64, FC], F32, tag="p64")
                        nc.tensor.matmul(p[:], fe_w2_sb[:], h1T[:, sl],
                                         start=True, stop=True)
                        nc.vector.tensor_scalar(h2T[:, sl], p[:], fe_b2_sb[:],
                                                0.0, OP.add, OP.max)
                    for fc in range(3):
                        sl = slice(FC * fc, FC * (fc + 1))
                        p = feps.tile([32, FC], F32, tag="p32")
                        nc.tensor.matmul(p[:], fe_w3_sb[:], h2T[:, sl],
                                         start=True, stop=True)
                        nc.vector.tensor_scalar(featsT[:, sl], p[:],
                                                fe_b3_sb[:], None, OP.add)
                    # A/B: [128, 264], partition 32r+c = (subject r, feat c)
                    pA = feps.tile([128, NNET], F32, tag="pAB")
                    pB = feps.tile([128, NNET], F32, tag="pAB")
                    for r in range(SPC):
                        fsl = featsT[:, NNET * r: NNET * (r + 1)]
                        nc.tensor.matmul(pA[32 * r: 32 * r + 32, :], smA_sb[:],
                                         fsl, start=True, stop=True,
                                         tile_position=(0, 32 * r))
                        nc.tensor.matmul(pB[32 * r: 32 * r + 32, :], smB_sb[:],
                                         fsl, start=True, stop=True,
                                         tile_position=(0, 32 * r))
                    nc.vector.tensor_scalar(A_sb[:], pA[:], smb1_sb[:],
                                            None, OP.add)
                    nc.vector.memset(B_ev[:, 264:266], 0.0)
                    nc.vector.memset(B_od[:, 263:266], 0.0)
                    nc.vector.tensor_copy(B_ev[:, 0:264], pB[:])
                    nc.vector.tensor_copy(B_od[:, 0:263], pB[:, 1:264])

                # ---------------- similarity MLP + classifier GEMM ----------
                with tc.tile_pool(name="accsb", bufs=1) as accsb_pool:
                    c1acc = accsb_pool.tile([B, COLS], F32)
                    nc.vector.memset(c1acc[:], 0.0)
                    with (
                        tc.tile_pool(name="h1p", bufs=4) as h1pool,
                        tc.tile_pool(name="h2p", bufs=4) as h2pool,
                        tc.tile_pool(name="h3p", bufs=4) as h3pool,
                        tc.tile_pool(name="sp", bufs=4) as spool,
                        tc.tile_pool(name="stp", bufs=2) as stpool,
                        tc.tile_pool(name="wp", bufs=W_BUFS) as wpool,
                        tc.tile_pool(name="ps2", bufs=2, space="PSUM") as ps2,
                        tc.tile_pool(name="ps3", bufs=2, space="PSUM") as ps3,
                        tc.tile_pool(name="ps4", bufs=2, space="PSUM") as ps4,
                        tc.tile_pool(name="psacc", bufs=2,
                                     space="PSUM") as psacc,
                    ):
                        def sim_tile(T):
                            h1p = h1pool.tile([128, 512], BF16)
                            for (a, b2, i, j0) in segs_by_tile[T]:
                                if j0 % 2 == 0:
                                    src = B_ev[:, j0: j0 + (b2 - a)]
                                else:
                                    src = B_od[:, j0 - 1: j0 - 1 + (b2 - a)]
                                nc.vector.tensor_scalar(
                                    h1p[:, a:b2], src, A_sb[:, i: i + 1],
                                    0.0, OP.add, OP.max)
                            if T == NT - 1:
                                nc.vector.memset(h1p[:, 32:512], 0.0)
                            p2 = ps2.tile([64, 512], F32)
                            nc.tensor.matmul(p2[:], w2bd_sb[:], h1p[:],
                                             start=True, stop=True)
                            h2p = h2pool.tile([64, 512], BF16)
                            if T % 2 == 0:
                                nc.scalar.activation(h2p[:], p2[:], AF.Relu,
                                                     bias=b2bd_sb[:])
                            else:
                                nc.vector.tensor_scalar(h2p[:], p2[:],
                                                        b2bd_sb[:],
                                                        0.0, OP.add, OP.max)
                            p3 = ps3.tile([32, 512], F32)
                            nc.tensor.matmul(p3[:], w3bd_sb[:], h2p[:],
                                             start=True, stop=True)
                            h3p = h3pool.tile([32, 512], BF16)
                            if T % 2 == 0:
                                nc.vector.tensor_scalar(h3p[:], p3[:],
                                                        b3bd_sb[:],
                                                        0.0, OP.add, OP.max)
                            else:
                                nc.scalar.activation(h3p[:], p3[:], AF.Relu,
                                                     bias=b3bd_sb[:])
                            p4 = ps4.tile([4, 512], F32)
                            nc.tensor.matmul(p4[:], w4bd_sb[:], h3p[:],
                                             start=True, stop=True)
                            s_bf = spool.tile([4, 512], BF16)
                            if T == NT - 1:
                                nc.scalar.activation(s_bf[:, 0:32],
                                                     p4[:, 0:32], AF.Tanh,
                                                     bias=b4bd_sb[:])
                                nc.vector.memset(s_bf[:, 32:512], 1.0)
                            else:
                                nc.scalar.activation(s_bf[:], p4[:], AF.Tanh,
                                                     bias=b4bd_sb[:])
                            return s_bf

                        def sim_chunk(c):
                            for tt in range(_TILES_PER_CHUNK[c]):
                                s_bf = sim_tile(_TILE0[c] + tt)
                                nc.sync.dma_start(
                                    s_loc[c][:, 512 * tt: 512 * (tt + 1)],
                                    s_bf[:])

                        def ag(c):
                            nc.gpsimd.collective_compute(
                                "AllGather", OP.bypass, replica_groups=RG,
                                ins=[s_loc[c][:]], outs=[s_full[c][:]])

                        def clf_chunk(c):
                            accp = psacc.tile([B, COLS], F32)
                            sT = stpool.tile(
                                [128, KPC[c], 32], BF16, tag="sT",
                                padded_shape=[128, max(KPC), 32])
                            nc.sync.dma_start_transpose(sT[:], s_full[c][:])
                            for g in range(KPC[c] // WB):
                                w1t = wpool.tile([128, WB, COLS], BF16)
                                r0 = 128 * (4 * _TILE0[c] + WB * g)
                                src = w1loc[r0: r0 + 128 * WB, :].rearrange(
                                    "(a p) n -> p a n", p=128)
                                nc.sync.dma_start(w1t[:], src)
                                for a in range(WB):
                                    j = WB * g + a
                                    nc.tensor.matmul(
                                        accp[:], sT[:, j, :], w1t[:, a, :],
                                        start=(j == 0),
                                        stop=(j == KPC[c] - 1))
                            nc.vector.tensor_tensor(c1acc[:], c1acc[:],
                                                    accp[:], OP.add)

                        for _c in range(NCHUNK):
                            with nc.named_scope(f"sim{_c}"):
                                sim_chunk(_c)
                                ag(_c)
                            if _c == 2:
                                with nc.named_scope("clf0"):
                                    clf_chunk(0)
                        for _c in range(1, NCHUNK):
                            with nc.named_scope(f"clf{_c}"):
                                clf_chunk(_c)

                    # ---------------- tail ----------------------------------
                    with (
                        nc.named_scope("tail"),
                        tc.tile_pool(name="tail", bufs=1) as tail,
                        tc.tile_pool(name="tailps", bufs=2,
                                     space="PSUM") as tailps,
                    ):
                        # local transpose + rrelu of c1 -> [128, 4, 32] bf16
                        c1Tsb = tail.tile([128, 4, B], BF16)
                        for a in range(4):
                            pT = tailps.tile([128, B], F32, tag="pTc1")
                            nc.tensor.transpose(
                                pT[:], c1acc[:, 128 * a: 128 * (a + 1)],
                                ident_sb[:])
                            tsl = tail.tile([128, B], F32, tag="tslc1",
                                            bufs=2)
                            nc.vector.tensor_scalar_mul(tsl[:], pT[:], SLOPE)
                            nc.vector.tensor_tensor(c1Tsb[:, a, :], pT[:],
                                                    tsl[:], OP.max)
                        bias_kt = tail.tile([128, B], BF16)
                        nc.vector.memset(bias_kt[:], 0.0)
                        nc.vector.memset(bias_kt[0:1, :], 1.0)
                        # partial c2 over local columns, then AllReduce
                        pc2 = tailps.tile([32, 256], F32, tag="pc2", bufs=1)
                        for kt in range(5):
                            lhsT = c1Tsb[:, kt, :] if kt < 4 else bias_kt[:]
                            nc.tensor.matmul(pc2[:], lhsT, w2loc_sb[:, kt, :],
                                             start=(kt == 0), stop=(kt == 4))
                        c2part = tail.tile([32, 256], F32)
                        nc.vector.tensor_copy(c2part[:], pc2[:])
                        nc.sync.dma_start(ar_in[:], c2part[:])
                        nc.gpsimd.collective_compute(
                            "AllReduce", OP.add, replica_groups=RG,
                            ins=[ar_in[:]], outs=[ar_out[:]])
                        c2sb = tail.tile([32, 256], F32)
                        nc.sync.dma_start(c2sb[:], ar_out[:])
                        c2r = tail.tile([32, 256], F32)
                        nc.vector.tensor_scalar(c2r[:], c2sb[:], 0.0, None,
                                                OP.max)
                        c2T = tail.tile([128, 3, 32], F32)
                        for j in range(2):
                            pT = tailps.tile([128, 32], F32, tag="pT")
                            nc.tensor.transpose(
                                pT[:], c2r[:, 128 * j: 128 * (j + 1)],
                                ident_sb[:])
                            nc.vector.tensor_copy(c2T[:, j, :], pT[:])
                        nc.vector.memset(c2T[:, 2, :], 0.0)
                        nc.vector.memset(c2T[0:1, 2, :], 1.0)

                        pc3 = tailps.tile([32, 64], F32, tag="pc3", bufs=1)
                        for kt in range(3):
                            nc.tensor.matmul(pc3[:], c2T[:, kt, :],
                                             w3aug_sb[:, kt, :],
                                             start=(kt == 0), stop=(kt == 2))
                        c3sb = tail.tile([32, 64], F32)
                        nc.vector.tensor_scalar(c3sb[:], pc3[:], 0.0, None,
                                                OP.max)
                        c3T = tail.tile([128, 32], F32)
                        nc.vector.memset(c3T[:], 0.0)
                        pT3 = tailps.tile([64, 32], F32, tag="pT3", bufs=1)
                        nc.tensor.transpose(pT3[:], c3sb[:], ident_sb[:])
                        nc.vector.tensor_copy(c3T[0:64, :], pT3[:])
                        nc.vector.memset(c3T[64:65, :], 1.0)

                        pc4 = tailps.tile([32, 3], F32, tag="pc4", bufs=1)
                        nc.tensor.matmul(pc4[:], c3T[:], w4aug_sb[:],
                                         start=True, stop=True)
                        c4sb = tail.tile([32, 3], F32)
                        nc.vector.tensor_copy(c4sb[:], pc4[:])

                        m = tail.tile([32, 1], F32)
                        nc.vector.tensor_reduce(m[:], c4sb[:],
                                                mybir.AxisListType.X, OP.max)
                        negm = tail.tile([32, 1], F32)
                        nc.vector.tensor_scalar_mul(negm[:], m[:], -1.0)
                        esb = tail.tile([32, 3], F32)
                        nc.scalar.activation(esb[:], c4sb[:], AF.Exp,
                                             bias=negm[:])
                        ssum = tail.tile([32, 1], F32)
                        nc.vector.tensor_reduce(ssum[:], esb[:],
                                                mybir.AxisListType.X, OP.add)
                        lse = tail.tile([32, 1], F32)
                        nc.scalar.activation(lse[:], ssum[:], AF.Ln)
                        outsb = tail.tile([32, 3], F32)
                        nc.vector.tensor_scalar(outsb[:], c4sb[:], m[:],
                                                lse[:], OP.subtract,
                                                OP.subtract)
                        nc.sync.dma_start(out[:], outsb[:])
    nc.compile()
    return nc


def _prepare(inputs):
    f32 = np.float32
    bf16 = ml_dtypes.bfloat16
    x = np.asarray(inputs["x"], f32)
    g = {k: np.asarray(v, f32) for k, v in inputs.items()}

    fe_w1p = np.zeros((DP, 64), f32)
    fe_w1p[:D] = g["fe_w1"]
    sm_w1 = g["sm_w1"]
    smb1t = np.tile(g["sm_b1"], SPC).reshape(128, 1)
    w2bd = np.zeros((128, 64), f32)
    w3bd = np.zeros((64, 32), f32)
    w4bd = np.zeros((32, 4), f32)
    for r in range(SPC):
        w2bd[32 * r: 32 * (r + 1), 16 * r: 16 * (r + 1)] = g["sm_w2"]
        w3bd[16 * r: 16 * (r + 1), 8 * r: 8 * (r + 1)] = g["sm_w3"]
        w4bd[8 * r: 8 * (r + 1), r: r + 1] = g["sm_w4"]
    b2bdt = np.tile(g["sm_b2"], SPC).reshape(64, 1)
    b3bdt = np.tile(g["sm_b3"], SPC).reshape(32, 1)
    b4bdt = np.full((4, 1), g["sm_b4"][0], f32)

    w3aug = np.zeros((384, 64), f32)
    w3aug[:256] = g["cl_w3"]
    w3aug[256] = g["cl_b3"]
    w4aug = np.zeros((128, 3), f32)
    w4aug[:64] = g["cl_w4"]
    w4aug[64] = g["cl_b4"]

    common = dict(
        fe_w1=fe_w1p.astype(bf16), fe_b1=g["fe_b1"].reshape(64, 1),
        fe_w2=g["fe_w2"].astype(bf16), fe_b2=g["fe_b2"].reshape(64, 1),
        fe_w3=g["fe_w3"].astype(bf16), fe_b3=g["fe_b3"].reshape(32, 1),
        smA=np.ascontiguousarray(sm_w1[:32]).astype(bf16),
        smB=np.ascontiguousarray(sm_w1[32:]).astype(bf16),
        smb1t=smb1t, w2bd=w2bd.astype(bf16), b2bdt=b2bdt,
        w3bd=w3bd.astype(bf16), b3bdt=b3bdt,
        w4bd=w4bd.astype(bf16), b4bdt=b4bdt,
        w3aug=w3aug, w4aug=w4aug,
    )

    cl_w1 = g["cl_w1"]
    cl_b1 = g["cl_b1"]
    cl_w2 = g["cl_w2"]
    cl_b2 = g["cl_b2"]
    offs = np.concatenate([[0], np.cumsum(
        [NNET - 1 - i for i in range(NNET - 1)])]).astype(np.int64)
    in_maps = []
    for k in range(NCORES):
        xT = np.zeros((DP, INST), bf16)
        xT[:D] = (x[SPC * k: SPC * (k + 1)].transpose(2, 0, 1)
                  .reshape(D, INST).astype(bf16))
        csl = slice(COLS * k, COLS * (k + 1))
        w1loc = np.zeros((PP, COLS), bf16)
        w1c = cl_w1[:, csl].astype(bf16)
        for (i, start, L, _Lp) in _BLOCKS:
            w1loc[start: start + L] = w1c[offs[i]: offs[i] + L]
        w1loc[BIAS_COL] = cl_b1[csl].astype(bf16)
        w2loc = np.zeros((640, 256), f32)
        w2loc[:COLS] = cl_w2[csl]
        w2loc[COLS + 0] = cl_b2 / NCORES
        m = dict(common)
        m["xT"] = xT
        m["w1loc"] = w1loc
        m["w2loc"] = w2loc.astype(bf16)
        in_maps.append(m)
    return in_maps


def kernel(**inputs):
    from concourse.bass_utils import run_bass_kernel_spmd

    if "nc" not in _CACHE:
        _CACHE["nc"] = _build()
    nc = _CACHE["nc"]
    in_maps = _prepare(inputs)
    res = run_bass_kernel_spmd(nc, in_maps, core_ids=list(range(NCORES)))
    _last["result"] = res
    return np.asarray(res.results[0]["out"], np.float32).copy()


if __name__ == "__main__":
    data = np.load("/root/problem/ref_inputs.npz")
    outv = kernel(**{k: data[k] for k in data.files})
    exp = np.load("/root/problem/ref_out.npy")
    err = np.abs(outv - exp).max() / np.abs(exp).max()
    print("rel err vs saved reference:", err)


# revision 13
# speedup vs baseline: 1.1381x; 1.1381x over previous
"""Trainium2 Bass kernel for nn_DeepFCNet (B=32 subjects, N=264 nets, D=375).

Strategy (8 NeuronCores, SPMD single NEFF):
  - Subject-data-parallel front: core k owns subjects 4k..4k+3. Feature
    extractor + similarity MLP for all 34716 pairs, 4-subject-packed into
    128 partitions via block-diagonal similarity weights (bf16 matmuls).
  - Pair layer 1 never materializes [P, 64]: with A = feats@sm_w1[:32]+b1
    and Bm = feats@sm_w1[32:], pre-activation of pair (i,j) is
    A[:, i] + Bm[:, j]; triu blocks make both reads affine (one DVE
    tensor_scalar per block segment). Blocks are padded to even column
    starts/lengths and Bm is kept in two one-column-shifted copies so
    every DVE op is 4-byte aligned (DVE 4x mode).
  - s ([4 x padded-pairs] tanh, bf16) is AllGathered in 4 chunks, batch
    DMA-transposed ([32, CH] -> [128, CH/128, 32]) and fed to a
    column-sharded classifier GEMM (cl_w1 column shard, bf16, padded rows
    zero). cl_b1 rides as an extra W row against a constant s=1 pad col.
  - c1 [32, 512] is transposed + rrelu'd locally (bf16); the first tail
    GEMM is computed on local columns only and AllReduced ([32, 256] fp32,
    cl_b2/8 folded per core); the rest of the tail (256-64-3 + log_softmax)
    runs redundantly on every core. End-to-end rel err ~2e-6.
"""

import os
import sys

for _p in ("/opt/trn_rl_repo", "/root/.axon_site/_ro/trn_rl_repo"):
    if os.path.isdir(_p) and _p not in sys.path:
        sys.path.insert(0, _p)

import numpy as np
import ml_dtypes

# ---------------------------------------------------------------- constants
NCORES = 8
B, NNET, D = 32, 264, 375
SPC = B // NCORES            # subjects per core = 4
DP = 384                     # padded feature dim (3 k-tiles)
INST = SPC * NNET            # 1056 instances per core through the FE
PAIRS = NNET * (NNET - 1) // 2   # 34716
COLS = 4096 // NCORES        # 512 classifier columns per core
SLOPE = 11.0 / 48.0
WB = 4                       # W k-tiles per DMA
W_BUFS = 24
NCHUNK = 4

_CACHE = {}
_last = {}


def _padded_blocks():
    """Blocks with even column starts and even padded lengths.
    Returns (blocks, used_cols): blocks = [(i, start, L, Lpad)]."""
    col, blocks = 0, []
    for i in range(NNET - 1):
        L = NNET - 1 - i
        Lp = L + (L & 1)
        blocks.append((i, col, L, Lp))
        col += Lp
    return blocks, col


_BLOCKS, _USED = _padded_blocks()        # used = 34848
PP = (_USED + 511) // 512 * 512          # padded pair columns = 35328
NT = PP // 512                           # 69 tiles
KT = PP // 128                           # 276 classifier k-tiles
_TILES_PER_CHUNK = [19, 19, 19, 12]
_TILE0 = [0, 19, 38, 57]
CH = [512 * t for t in _TILES_PER_CHUNK]          # chunk pair cols
KPC = [4 * t for t in _TILES_PER_CHUNK]           # chunk k-tiles
BIAS_COL = _USED                         # s=1 / cl_b1 row lives here


def _segments_by_tile():
    """Per 512-wide pair tile: list of (a, b, i, j0) meaning
    h1[:, a:b] = relu(Bm[:, j0 : j0+(b-a)] + A[:, i]). a, b-a even."""
    segs = [[] for _ in range(NT)]
    for (i, start, L, Lp) in _BLOCKS:
        s0, s1 = start, start + Lp
        for T in range(s0 // 512, (s1 - 1) // 512 + 1):
            lo, hi = max(s0, 512 * T), min(s1, 512 * (T + 1))
            segs[T].append((lo - 512 * T, hi - 512 * T, i, i + 1 + (lo - s0)))
    return segs


def _build():
    import concourse.tile as tile
    from concourse import bacc, mybir
    from concourse.masks import make_identity

    F32 = mybir.dt.float32
    BF16 = mybir.dt.bfloat16
    AF = mybir.ActivationFunctionType
    OP = mybir.AluOpType

    nc = bacc.Bacc(None, target_bir_lowering=False, debug=False,
                   num_devices=NCORES)
    segs_by_tile = _segments_by_tile()
    RG = [list(range(NCORES))]

    with tile.TileContext(nc) as tc:
        with tc.tile_pool(name="dram", bufs=1, space="DRAM") as dram:
            def din(name, shape, dt=F32):
                return dram.tile(shape, dt, kind="ExternalInput", name=name,
                                 uniquify=False)

            xT = din("xT", [DP, INST], BF16)
            fe_w1 = din("fe_w1", [DP, 64], BF16)
            fe_b1 = din("fe_b1", [64, 1])
            fe_w2 = din("fe_w2", [64, 64], BF16)
            fe_b2 = din("fe_b2", [64, 1])
            fe_w3 = din("fe_w3", [64, 32], BF16)
            fe_b3 = din("fe_b3", [32, 1])
            smA = din("smA", [32, 32], BF16)
            smB = din("smB", [32, 32], BF16)
            smb1t = din("smb1t", [128, 1])
            w2bd = din("w2bd", [128, 64], BF16)
            b2bdt = din("b2bdt", [64, 1])
            w3bd = din("w3bd", [64, 32], BF16)
            b3bdt = din("b3bdt", [32, 1])
            w4bd = din("w4bd", [32, 4], BF16)
            b4bdt = din("b4bdt", [4, 1])
            w1loc = din("w1loc", [PP, COLS], BF16)
            w2loc = din("w2loc", [640, 256], BF16)   # local-col rows + b2/8
            w3aug = din("w3aug", [384, 64])
            w4aug = din("w4aug", [128, 3])
            out = dram.tile([B, 3], F32, kind="ExternalOutput", name="out",
                            uniquify=False)

            warm_in = dram.tile([8, 8], F32, name="warm_in")
            scr_sink = dram.tile([4, 4], F32, name="scr_sink")
            warm_out = dram.tile([64, 8], F32, addr_space="Shared",
                                 name="warm_out")
            s_loc = [dram.tile([SPC, CH[c]], BF16, name=f"s_loc{c}")
                     for c in range(NCHUNK)]
            s_full = [dram.tile([B, CH[c]], BF16, addr_space="Shared",
                                name=f"s_full{c}") for c in range(NCHUNK)]
            ar_in = dram.tile([B, 256], F32, name="ar_in")
            ar_out = dram.tile([B, 256], F32, addr_space="Shared",
                               name="ar_out")

            with tc.tile_pool(name="const", bufs=1) as const:
                # fire a tiny collective first: absorbs first-call setup
                nc.gpsimd.collective_compute(
                    "AllGather", OP.bypass, replica_groups=RG,
                    ins=[warm_in[:]], outs=[warm_out[:]])

                def ld(name, shape, src, dt=F32):
                    t = const.tile(shape, dt, name=name)
                    nc.scalar.dma_start(t[:], src)
                    return t

                fe_w1_sb = ld("fe_w1_sb", [128, 3, 64],
                              fe_w1.rearrange("(t p) m -> p t m", p=128), BF16)
                fe_b1_sb = ld("fe_b1_sb", [64, 1], fe_b1[:])
                fe_w2_sb = ld("fe_w2_sb", [64, 64], fe_w2[:], BF16)
                fe_b2_sb = ld("fe_b2_sb", [64, 1], fe_b2[:])
                fe_w3_sb = ld("fe_w3_sb", [64, 32], fe_w3[:], BF16)
                fe_b3_sb = ld("fe_b3_sb", [32, 1], fe_b3[:])
                smA_sb = ld("smA_sb", [32, 32], smA[:], BF16)
                smB_sb = ld("smB_sb", [32, 32], smB[:], BF16)
                smb1_sb = ld("smb1_sb", [128, 1], smb1t[:])
                w2bd_sb = ld("w2bd_sb", [128, 64], w2bd[:], BF16)
                b2bd_sb = ld("b2bd_sb", [64, 1], b2bdt[:])
                w3bd_sb = ld("w3bd_sb", [64, 32], w3bd[:], BF16)
                b3bd_sb = ld("b3bd_sb", [32, 1], b3bdt[:])
                w4bd_sb = ld("w4bd_sb", [32, 4], w4bd[:], BF16)
                b4bd_sb = ld("b4bd_sb", [4, 1], b4bdt[:])
                w2loc_sb = ld("w2loc_sb", [128, 5, 256],
                              w2loc.rearrange("(t p) m -> p t m", p=128), BF16)
                w3aug_sb = ld("w3aug_sb", [128, 3, 64],
                              w3aug.rearrange("(t p) m -> p t m", p=128))
                w4aug_sb = ld("w4aug_sb", [128, 3], w4aug[:])
                ident_sb = const.tile([32, 32], F32, name="ident_sb")
                make_identity(nc, ident_sb[:])
                # pre-load ACT tables (Tanh/Relu/Exp/Ln) off the critical path
                scr = const.tile([32, 4], F32, name="scr")
                nc.scalar.activation(scr[:, 0:1], ident_sb[:, 0:1], AF.Tanh)
                nc.scalar.activation(scr[:, 1:2], ident_sb[:, 0:1], AF.Relu)
                nc.scalar.activation(scr[:, 2:3], ident_sb[:, 0:1], AF.Exp)
                nc.scalar.activation(scr[:, 3:4], scr[:, 2:3], AF.Ln)
                nc.scalar.dma_start(scr_sink[:], scr[0:4, :])
                A_sb = const.tile([128, NNET], F32)      # f32: scalar operand
                B_ev = const.tile([128, 266], BF16)
                B_od = const.tile([128, 266], BF16)

                # ---------------- feature extractor ------------------------
                with (
                    nc.named_scope("fe"),
                    tc.tile_pool(name="fe", bufs=1) as fe,
                    tc.tile_pool(name="fetmp", bufs=2) as fetmp,
                    tc.tile_pool(name="feps", bufs=2, space="PSUM") as feps,
                ):
                    xT_sb = fe.tile([128, 3, INST], BF16)
                    nc.scalar.dma_start(xT_sb[:],
                                        xT.rearrange("(t p) i -> p t i",
                                                     p=128))
                    h1T = fe.tile([64, INST], BF16)
                    h2T = fe.tile([64, INST], BF16)
                    featsT = fe.tile([32, INST], BF16)
                    FC = INST // 3  # 352
                    for fc in range(3):
                        sl = slice(FC * fc, FC * (fc + 1))
                        p = feps.tile([64, FC], F32, tag="p64")
                        for t in range(3):
                            nc.tensor.matmul(p[:], fe_w1_sb[:, t, :],
                                             xT_sb[:, t, sl],
                                             start=(t == 0), stop=(t == 2))
                        tmp = fetmp.tile([64, FC], F32)
                        nc.vector.tensor_scalar(tmp[:], p[:], fe_b1_sb[:],
                                                None, OP.add)
                        nc.vector.scalar_tensor_tensor(
                            h1T[:, sl], tmp[:], SLOPE, tmp[:], OP.mult, OP.max)
                    for fc in range(3):
                        sl = slice(FC * fc, FC * (fc + 1))
                        p = feps.tile([64, FC], F32, tag="p64")
                        nc.tensor.matmul(p[:], fe_w2_sb[:], h1T[:, sl],
                                         start=True, stop=True)
                        nc.vector.tensor_scalar(h2T[:, sl], p[:], fe_b2_sb[:],
                                                0.0, OP.add, OP.max)
                    for fc in range(3):
                        sl = slice(FC * fc, FC * (fc + 1))
                        p = feps.tile([32, FC], F32, tag="p32")
                        nc.tensor.matmul(p[:], fe_w3_sb[:], h2T[:, sl],
                                         start=True, stop=True)
                        nc.vector.tensor_scalar(featsT[:, sl], p[:],
                                                fe_b3_sb[:], None, OP.add)
                    # A/B: [128, 264], partition 32r+c = (subject r, feat c)
                    pA = feps.tile([128, NNET], F32, tag="pAB")
                    pB = feps.tile([128, NNET], F32, tag="pAB")
                    for r in range(SPC):
                        fsl = featsT[:, NNET * r: NNET * (r + 1)]
                        nc.tensor.matmul(pA[32 * r: 32 * r + 32, :], smA_sb[:],
                                         fsl, start=True, stop=True,
                                         tile_position=(0, 32 * r))
                        nc.tensor.matmul(pB[32 * r: 32 * r + 32, :], smB_sb[:],
                                         fsl, start=True, stop=True,
                                         tile_position=(0, 32 * r))
                    nc.vector.tensor_scalar(A_sb[:], pA[:], smb1_sb[:],
                                            None, OP.add)
                    nc.vector.memset(B_ev[:, 264:266], 0.0)
                    nc.vector.memset(B_od[:, 263:266], 0.0)
                    nc.vector.tensor_copy(B_ev[:, 0:264], pB[:])
                    nc.vector.tensor_copy(B_od[:, 0:263], pB[:, 1:264])

                # ---------------- similarity MLP + classifier GEMM ----------
                with tc.tile_pool(name="accsb", bufs=1) as accsb_pool:
                    c1acc = accsb_pool.tile([B, COLS], F32)
                    nc.vector.memset(c1acc[:], 0.0)
                    with (
                        tc.tile_pool(name="h1p", bufs=4) as h1pool,
                        tc.tile_pool(name="h2p", bufs=4) as h2pool,
                        tc.tile_pool(name="h3p", bufs=4) as h3pool,
                        tc.tile_pool(name="sp", bufs=4) as spool,
                        tc.tile_pool(name="stp", bufs=2) as stpool,
                        tc.tile_pool(name="wp", bufs=W_BUFS) as wpool,
                        tc.tile_pool(name="ps2", bufs=2, space="PSUM") as ps2,
                        tc.tile_pool(name="ps3", bufs=2, space="PSUM") as ps3,
                        tc.tile_pool(name="ps4", bufs=2, space="PSUM") as ps4,
                        tc.tile_pool(name="psacc", bufs=2,
                                     space="PSUM") as psacc,
                    ):
                        def sim_tile(T):
                            h1p = h1pool.tile([128, 512], BF16)
                            for (a, b2, i, j0) in segs_by_tile[T]:
                                if j0 % 2 == 0:
                                    src = B_ev[:, j0: j0 + (b2 - a)]
                                else:
                                    src = B_od[:, j0 - 1: j0 - 1 + (b2 - a)]
                                nc.vector.tensor_scalar(
                                    h1p[:, a:b2], src, A_sb[:, i: i + 1],
                                    0.0, OP.add, OP.max)
                            if T == NT - 1:
                                nc.vector.memset(h1p[:, 32:512], 0.0)
                            p2 = ps2.tile([64, 512], F32)
                            nc.tensor.matmul(p2[:], w2bd_sb[:], h1p[:],
                                             start=True, stop=True)
                            h2p = h2pool.tile([64, 512], BF16)
                            if T % 2 == 0:
                                nc.scalar.activation(h2p[:], p2[:], AF.Relu,
                                                     bias=b2bd_sb[:])
                            else:
                                nc.vector.tensor_scalar(h2p[:], p2[:],
                                                        b2bd_sb[:],
                                                        0.0, OP.add, OP.max)
                            p3 = ps3.tile([32, 512], F32)
                            nc.tensor.matmul(p3[:], w3bd_sb[:], h2p[:],
                                             start=True, stop=True)
                            h3p = h3pool.tile([32, 512], BF16)
                            if T % 2 == 0:
                                nc.vector.tensor_scalar(h3p[:], p3[:],
                                                        b3bd_sb[:],
                                                        0.0, OP.add, OP.max)
                            else:
                                nc.scalar.activation(h3p[:], p3[:], AF.Relu,
                                                     bias=b3bd_sb[:])
                            p4 = ps4.tile([4, 512], F32)
                            nc.tensor.matmul(p4[:], w4bd_sb[:], h3p[:],
                                             start=True, stop=True)
                            s_bf = spool.tile([4, 512], BF16)
                            if T == NT - 1:
                                nc.scalar.activation(s_bf[:, 0:32],
                                                     p4[:, 0:32], AF.Tanh,
                                                     bias=b4bd_sb[:])
                                nc.vector.memset(s_bf[:, 32:512], 1.0)
                            else:
                                nc.scalar.activation(s_bf[:], p4[:], AF.Tanh,
                                                     bias=b4bd_sb[:])
                            return s_bf

                        def sim_chunk(c):
                            for tt in range(_TILES_PER_CHUNK[c]):
                                s_bf = sim_tile(_TILE0[c] + tt)
                                nc.sync.dma_start(
                                    s_loc[c][:, 512 * tt: 512 * (tt + 1)],
                                    s_bf[:])

                        def ag(c):
                            nc.gpsimd.collective_compute(
                                "AllGather", OP.bypass, replica_groups=RG,
                                ins=[s_loc[c][:]], outs=[s_full[c][:]])

                        def clf_chunk(c):
                            accp = psacc.tile([B, COLS], F32)
                            sT = stpool.tile(
                                [128, KPC[c], 32], BF16, tag="sT",
                                padded_shape=[128, max(KPC), 32])
                            nc.sync.dma_start_transpose(sT[:], s_full[c][:])
                            for g in range(KPC[c] // WB):
                                w1t = wpool.tile([128, WB, COLS], BF16)
                                r0 = 128 * (4 * _TILE0[c] + WB * g)
                                src = w1loc[r0: r0 + 128 * WB, :].rearrange(
                                    "(a p) n -> p a n", p=128)
                                nc.sync.dma_start(w1t[:], src)
                                for a in range(WB):
                                    j = WB * g + a
                                    nc.tensor.matmul(
                                        accp[:], sT[:, j, :], w1t[:, a, :],
                                        start=(j == 0),
                                        stop=(j == KPC[c] - 1))
                            nc.vector.tensor_tensor(c1acc[:], c1acc[:],
                                                    accp[:], OP.add)

                        for _c in range(NCHUNK):
                            with nc.named_scope(f"sim{_c}"):
                                sim_chunk(_c)
                                ag(_c)
                            if _c == 2:
                                with nc.named_scope("clf0"):
                                    clf_chunk(0)
                        for _c in range(1, NCHUNK):
                            with nc.named_scope(f"clf{_c}"):
                                clf_chunk(_c)

                    # ---------------- tail ----------------------------------
                    with (
                        nc.named_scope("tail"),
                        tc.tile_pool(name="tail", bufs=1) as tail,
                        tc.tile_pool(name="tailps", bufs=2,
                                     space="PSUM") as tailps,
                    ):
                        # local transpose + rrelu of c1 -> [128, 4, 32] bf16
                        c1Tsb = tail.tile([128, 4, B], BF16)
                        for a in range(4):
                            pT = tailps.tile([128, B], F32, tag="pTc1")
                            nc.tensor.transpose(
                                pT[:], c1acc[:, 128 * a: 128 * (a + 1)],
                                ident_sb[:])
                            tsl = tail.tile([128, B], F32, tag="tslc1",
                                            bufs=2)
                            nc.vector.tensor_scalar_mul(tsl[:], pT[:], SLOPE)
                            nc.vector.tensor_tensor(c1Tsb[:, a, :], pT[:],
                                                    tsl[:], OP.max)
                        bias_kt = tail.tile([128, B], BF16)
                        nc.vector.memset(bias_kt[:], 0.0)
                        nc.vector.memset(bias_kt[0:1, :], 1.0)
                        # partial c2 over local columns, then AllReduce
                        pc2 = tailps.tile([32, 256], F32, tag="pc2", bufs=1)
                        for kt in range(5):
                            lhsT = c1Tsb[:, kt, :] if kt < 4 else bias_kt[:]
                            nc.tensor.matmul(pc2[:], lhsT, w2loc_sb[:, kt, :],
                                             start=(kt == 0), stop=(kt == 4))
                        c2part = tail.tile([32, 256], F32)
                        nc.vector.tensor_copy(c2part[:], pc2[:])
                        nc.sync.dma_start(ar_in[:], c2part[:])
                        nc.gpsimd.collective_compute(
                            "AllReduce", OP.add, replica_groups=RG,
                            ins=[ar_in[:]], outs=[ar_out[:]])
                        c2sb = tail.tile([32, 256], F32)
                        nc.sync.dma_start(c2sb[:], ar_out[:])
                        c2r = tail.tile([32, 256], F32)
                        nc.vector.tensor_scalar(c2r[:], c2sb[:], 0.0, None,
                                                OP.max)
                        c2T = tail.tile([128, 3, 32], F32)
                        for j in range(2):
                            pT = tailps.tile([128, 32], F32, tag="pT")
                            nc.tensor.transpose(
                                pT[:], c2r[:, 128 * j: 128 * (j + 1)],
                                ident_sb[:])
                            nc.vector.tensor_copy(c2T[:, j, :], pT[:])
                        nc.vector.memset(c2T[:, 2, :], 0.0)
                        nc.vector.memset(c2T[0:1, 2, :], 1.0)

                        pc3 = tailps.tile([32, 64], F32, tag="pc3", bufs=1)
                        for kt in range(3):
                            nc.tensor.matmul(pc3[:], c2T[:, kt, :],
                                             w3aug_sb[:, kt, :],
                                             start=(kt == 0), stop=(kt == 2))
                        c3sb = tail.tile([32, 64], F32)
                        nc.vector.tensor_scalar(c3sb[:], pc3[:], 0.0, None,
                                                OP.max)
                        c3T = tail.tile([128, 32], F32)
                        nc.vector.memset(c3T[:], 0.0)
                        pT3 = tailps.tile([64, 32], F32, tag="pT3", bufs=1)
                        nc.tensor.transpose(pT3[:], c3sb[:], ident_sb[:])
                        nc.vector.tensor_copy(c3T[0:64, :], pT3[:])
                        nc.vector.memset(c3T[64:65, :], 1.0)

                        pc4 = tailps.tile([32, 3], F32, tag="pc4", bufs=1)
                        nc.tensor.matmul(pc4[:], c3T[:], w4aug_sb[:],
                                         start=True, stop=True)
                        c4sb = tail.tile([32, 3], F32)
                        nc.vector.tensor_copy(c4sb[:], pc4[:])

                        m = tail.tile([32, 1], F32)
                        nc.vector.tensor_reduce(m[:], c4sb[:],
                                                mybir.AxisListType.X, OP.max)
                        negm = tail.tile([32, 1], F32)
                        nc.vector.tensor_scalar_mul(negm[:], m[:], -1.0)
                        esb = tail.tile([32, 3], F32)
                        nc.scalar.activation(esb[:], c4sb[:], AF.Exp,
                                             bias=negm[:])
                        ssum = tail.tile([32, 1], F32)
                        nc.vector.tensor_reduce(ssum[:], esb[:],
                                                mybir.AxisListType.X, OP.add)
                        lse = tail.tile([32, 1], F32)
                        nc.scalar.activation(lse[:], ssum[:], AF.Ln)
                        outsb = tail.tile([32, 3], F32)
                        nc.vector.tensor_scalar(outsb[:], c4sb[:], m[:],
                                                lse[:], OP.subtract,
                                                OP.subtract)
                        nc.sync.dma_start(out[:], outsb[:])
    nc.compile()
    return nc


def _prepare(inputs):
    f32 = np.float32
    bf16 = ml_dtypes.bfloat16
    x = np.asarray(inputs["x"], f32)
    g = {k: np.asarray(v, f32) for k, v in inputs.items()}

    fe_w1p = np.zeros((DP, 64), f32)
    fe_w1p[:D] = g["fe_w1"]
    sm_w1 = g["sm_w1"]
    smb1t = np.tile(g["sm_b1"], SPC).reshape(128, 1)
    w2bd = np.zeros((128, 64), f32)
    w3bd = np.zeros((64, 32), f32)
    w4bd = np.zeros((32, 4), f32)
    for r in range(SPC):
        w2bd[32 * r: 32 * (r + 1), 16 * r: 16 * (r + 1)] = g["sm_w2"]
        w3bd[16 * r: 16 * (r + 1), 8 * r: 8 * (r + 1)] = g["sm_w3"]
        w4bd[8 * r: 8 * (r + 1), r: r + 1] = g["sm_w4"]
    b2bdt = np.tile(g["sm_b2"], SPC).reshape(64, 1)
    b3bdt = np.tile(g["sm_b3"], SPC).reshape(32, 1)
    b4bdt = np.full((4, 1), g["sm_b4"][0], f32)

    w3aug = np.zeros((384, 64), f32)
    w3aug[:256] = g["cl_w3"]
    w3aug[256] = g["cl_b3"]
    w4aug = np.zeros((128, 3), f32)
    w4aug[:64] = g["cl_w4"]
    w4aug[64] = g["cl_b4"]

    common = dict(
        fe_w1=fe_w1p.astype(bf16), fe_b1=g["fe_b1"].reshape(64, 1),
        fe_w2=g["fe_w2"].astype(bf16), fe_b2=g["fe_b2"].reshape(64, 1),
        fe_w3=g["fe_w3"].astype(bf16), fe_b3=g["fe_b3"].reshape(32, 1),
        smA=np.ascontiguousarray(sm_w1[:32]).astype(bf16),
        smB=np.ascontiguousarray(sm_w1[32:]).astype(bf16),
        smb1t=smb1t, w2bd=w2bd.astype(bf16), b2bdt=b2bdt,
        w3bd=w3bd.astype(bf16), b3bdt=b3bdt,
        w4bd=w4bd.astype(bf16), b4bdt=b4bdt,
        w3aug=w3aug, w4aug=w4aug,
    )

    cl_w1 = g["cl_w1"]
    cl_b1 = g["cl_b1"]
    cl_w2 = g["cl_w2"]
    cl_b2 = g["cl_b2"]
    offs = np.concatenate([[0], np.cumsum(
        [NNET - 1 - i for i in range(NNET - 1)])]).astype(np.int64)
    in_maps = []
    for k in range(NCORES):
        xT = np.zeros((DP, INST), bf16)
        xT[:D] = (x[SPC * k: SPC * (k + 1)].transpose(2, 0, 1)
                  .reshape(D, INST).astype(bf16))
        csl = slice(COLS * k, COLS * (k + 1))
        w1loc = np.zeros((PP, COLS), bf16)
        w1c = cl_w1[:, csl].astype(bf16)
        for (i, start, L, _Lp) in _BLOCKS:
            w1loc[start: start + L] = w1c[offs[i]: offs[i] + L]
        w1loc[BIAS_COL] = cl_b1[csl].astype(bf16)
        w2loc = np.zeros((640, 256), f32)
        w2loc[:COLS] = cl_w2[csl]
        w2loc[COLS + 0] = cl_b2 / NCORES
        m = dict(common)
        m["xT"] = xT
        m["w1loc"] = w1loc
        m["w2loc"] = w2loc.astype(bf16)
        in_maps.append(m)
    return in_maps


def kernel(**inputs):
    from concourse.bass_utils import run_bass_kernel_spmd

    if "nc" not in _CACHE:
        _CACHE["nc"] = _build()
    nc = _CACHE["nc"]
    in_maps = _prepare(inputs)
    res = run_bass_kernel_spmd(nc, in_maps, core_ids=list(range(NCORES)))
    _last["result"] = res
    return np.asarray(res.results[0]["out"], np.float32).copy()


if __name__ == "__main__":
    data = np.load("/root/problem/ref_inputs.npz")
    outv = kernel(**{k: data[k] for k in data.files})
    exp = np.load("/root/problem/ref_out.npy")
    err = np.abs(outv - exp).max() / np.abs(exp).max()
    print("rel err vs saved reference:", err)
